# revision 2
# baseline (speedup 1.0000x reference)
"""Causal single-head attention (B=4, S=4096, D_MODEL=1024, D_K=D_V=128)
distributed over 8 TRN2 NeuronCores.

Sharding: batch (4) x interleaved query-tile parity (2) = 8 cores.
Core c handles batch b=c//2, parity p=c%2; its local q-tile i (16 tiles of
128 queries) is global q-tile T = 2*i + p.  The causal workload per-core is
identical (sum over i of (2i+2) key-tiles), so one SPMD program serves all
cores; the parity difference lives in two host-supplied [128,128] masks.

v2: each core projects only its parity's half of the keys (even core: keys
[0:2048), odd: [2048:4096)) and the projected K^T / V tiles are exchanged
within each batch pair by two AllGather collectives (0.5 MB each), halving
both the k/v HBM traffic and the K/V projection FLOPs.

Per-core compute layout ("all transposed", no PE transposes):
  - projections produce QT [dk=128, q], KT [dk=128, keys] (N=512 matmuls)
    and V [keys, dv] (N=128 matmuls)
  - scores ST = [keys=128, q<=512] tiles, two key-tiles per PSUM tile so
    one ACT exp covers a pair (ACT ops pay (N+352)/1.2 ns)
  - softmax: exp on ScalarE (scores bounded ~|z|<3: no max subtraction),
    causal via binary mask multiplies on VectorE
  - PV: matmul(lhsT=P_tile [keys,128q], rhs=V_aug [keys,129]) -> output in
    natural [q, dv] layout with the softmax denominator in column 128;
    normalization = DVE reciprocal + tensor_scalar_mul.
"""

import math
import numpy as np
import ml_dtypes

import concourse.bass as bass
import concourse.mybir as mybir
from concourse import bacc, tile
from concourse.bass_utils import run_bass_kernel_spmd

BF16NP = ml_dtypes.bfloat16
F32 = mybir.dt.float32
BF16 = mybir.dt.bfloat16

B = 4
S = 4096
DM = 1024
DK = 128
DV = 128
SQ = 2048          # queries per core
NQT = 16           # local q-tiles of 128
NMC = DM // 128    # 8 contraction chunks for projections
MAXKT = S // 128   # 32 key tiles
NCH = SQ // 512    # 4 q-chunks of 512
SK = S // 2        # keys projected locally per core (v2)

MODE = "v2"        # "v2" (collective) | "v15" (full local K/V)

LAST_RESULTS = None
_NC_CACHE = {}


def build_nc(mode="v2", vt=False):
    collective = mode == "v2"
    nkeys = SK if collective else S

    nc = bacc.Bacc(None, target_bir_lowering=False, num_devices=8)

    qT = nc.declare_dram_parameter("qT", [DM, SQ], BF16, isOutput=False)
    kT = nc.declare_dram_parameter("kT", [DM, nkeys], BF16, isOutput=False)
    vT = nc.declare_dram_parameter("vT", [DM, nkeys], BF16, isOutput=False)
    wq = nc.declare_dram_parameter("wq", [128, NMC * DK], BF16, isOutput=False)
    wk = nc.declare_dram_parameter("wk", [128, NMC * DK], BF16, isOutput=False)
    wv = nc.declare_dram_parameter("wv", [128, NMC * DV], BF16, isOutput=False)
    mska = nc.declare_dram_parameter("mska", [128, 128], BF16, isOutput=False)
    mskb = nc.declare_dram_parameter("mskb", [128, 128], BF16, isOutput=False)
    eye = nc.declare_dram_parameter("eye", [128, 128], BF16, isOutput=False)
    out = nc.declare_dram_parameter("out", [SQ, DV], F32, isOutput=True)

    Exp = mybir.ActivationFunctionType.Exp

    with tile.TileContext(nc) as tc:
        with (
            tc.tile_pool(name="const", bufs=1) as constp,
            tc.tile_pool(name="stream", bufs=3) as streamp,
            tc.tile_pool(name="big", bufs=1) as bigp,
            tc.tile_pool(name="ptp", bufs=2) as ptp,
            tc.tile_pool(name="outp", bufs=4) as outp,
            tc.tile_pool(name="dram", bufs=1, space="DRAM") as dramp,
            tc.tile_pool(name="ps", bufs=2, space="PSUM") as psp,
            tc.tile_pool(name="pst", bufs=2, space="PSUM") as pstp,
            tc.tile_pool(name="pso", bufs=2, space="PSUM") as psop,
        ):
            # ---- weights (host pre-tiled to [128, mc*128] contiguous) ----
            wk_sb = constp.tile([128, NMC, DK], BF16)
            nc.sync.dma_start(wk_sb[:], wk.rearrange("p (mc d) -> p mc d", d=DK))
            wv_sb = constp.tile([128, NMC, DV], BF16)
            nc.sync.dma_start(wv_sb[:], wv.rearrange("p (mc d) -> p mc d", d=DV))
            wq_sb = constp.tile([128, NMC, DK], BF16)
            nc.sync.dma_start(wq_sb[:], wq.rearrange("p (mc d) -> p mc d", d=DK))

            # ---- PE warm-up: keep TensorE busy during the input-DMA lead-in
            # so HAM is at 8/8 when the first real matmul's data lands ----
            warm = constp.tile([128, 256], BF16)
            nc.vector.memset(warm[:], 0.0)
            wps = psp.tile([128, 256], F32, tag="projps")
            for _ in range(50):
                nc.tensor.matmul(wps[:], warm[:, 0:128], warm[:], start=True, stop=True)

            # ---- persistent activations ----
            QT = bigp.tile([128, SQ], BF16)
            KT = bigp.tile([128, S], BF16)
            VA = bigp.tile([128, MAXKT, DV + 2], BF16)

            # ---- projection helper: streams src in quarters of 1024 cols ----
            def proj_quarters(src_dram, n_cols):
                for qtr in range(n_cols // 1024):
                    tin = streamp.tile([128, NMC, 1024], BF16, tag="instream")
                    nc.sync.dma_start(
                        tin[:],
                        src_dram[:, qtr * 1024:(qtr + 1) * 1024].rearrange(
                            "(mc p) c -> p mc c", p=128
                        ),
                    )
                    yield qtr, tin

            def proj512(w_sb, tin, half, dst_sb_slice):
                ps = psp.tile([128, 512], F32, tag="projps")
                for m in range(NMC):
                    nc.tensor.matmul(
                        ps[:], w_sb[:, m, :], tin[:, m, half * 512:(half + 1) * 512],
                        start=(m == 0), stop=(m == NMC - 1),
                    )
                nc.vector.tensor_copy(dst_sb_slice, ps[:])

            # ---- Q projection first (non-collective): measured best PE order.
            # First two chunks are 512 cols so the very first matmul's data
            # arrives sooner. ----
            if not collective:
                cb = 0
                for w in (512, 512, 1024):
                    tin = streamp.tile([128, NMC, w], BF16, tag="instream",
                                       name=f"qin{cb}")
                    nc.sync.dma_start(
                        tin[:],
                        qT[:, cb * 512:cb * 512 + w].rearrange(
                            "(mc p) c -> p mc c", p=128
                        ),
                    )
                    for half in range(w // 512):
                        proj512(wq_sb, tin, half, QT[:, (cb + half) * 512:(cb + half + 1) * 512])
                    cb += w // 512

            # ---- K projection (local keys) ----
            if collective:
                KT_loc = bigp.tile([128, SK], BF16)
                k_dst = KT_loc
            else:
                k_dst = KT
            for qtr, tin in proj_quarters(kT, nkeys):
                for half in range(2):
                    cb = qtr * 2 + half
                    proj512(wk_sb, tin, half, k_dst[:, cb * 512:(cb + 1) * 512])

            if collective:
                cc_in_k = dramp.tile([128, SK], BF16)
                cc_out_k = dramp.tile([2, 128, SK], BF16)
                nc.sync.dma_start(cc_in_k[:], KT_loc[:])
                nc.gpsimd.collective_compute(
                    "AllGather",
                    mybir.AluOpType.bypass,
                    replica_groups=[[0, 1], [2, 3], [4, 5], [6, 7]],
                    ins=[cc_in_k[:]],
                    outs=[cc_out_k[:]],
                )

            # ---- V projection (local keys, natural [keys, dv] layout) ----
            nc.vector.memset(VA[:], 1.0)  # ones column at [:, :, DV]
            nloc_kt = nkeys // 128
            if collective:
                V_loc = bigp.tile([128, nloc_kt, DV], BF16)
            if vt and not collective:
                # N=512 VT projection + PE-mode transpose into VA tiles,
                # interleaved per 512-key chunk so HAM stays warm
                eye_sb = constp.tile([128, 128], BF16)
                nc.sync.dma_start(eye_sb[:], eye[:])
                VTS = bigp.tile([128, S], BF16)
                for qtr, tin in proj_quarters(vT, nkeys):
                    for half in range(2):
                        cb = qtr * 2 + half
                        proj512(wv_sb, tin, half, VTS[:, cb * 512:(cb + 1) * 512])
                        for sl in range(4):
                            kt_idx = cb * 4 + sl
                            tp = psop.tile([128, 128], BF16, tag="ops")
                            nc.tensor.transpose(
                                tp[:], VTS[:, kt_idx * 128:(kt_idx + 1) * 128], eye_sb[:]
                            )
                            nc.vector.tensor_copy(VA[:, kt_idx, 0:DV], tp[:])
            else:
                for qtr, tin in proj_quarters(vT, nkeys):
                    for sl in range(8):
                        kt_idx = qtr * 8 + sl
                        vps = psp.tile([128, DV], F32, tag="projps")
                        for m in range(NMC):
                            nc.tensor.matmul(
                                vps[:], tin[:, m, sl * 128:(sl + 1) * 128], wv_sb[:, m, :],
                                start=(m == 0), stop=(m == NMC - 1),
                            )
                        if collective:
                            nc.vector.tensor_copy(V_loc[:, kt_idx, :], vps[:])
                        else:
                            nc.vector.tensor_copy(VA[:, kt_idx, 0:DV], vps[:])

            if collective:
                cc_in_v = dramp.tile([128, SK], BF16)
                cc_out_v = dramp.tile([2, 128, SK], BF16)
                nc.sync.dma_start(cc_in_v.rearrange("p (kt d) -> p kt d", d=DV), V_loc[:])
                nc.gpsimd.collective_compute(
                    "AllGather",
                    mybir.AluOpType.bypass,
                    replica_groups=[[0, 1], [2, 3], [4, 5], [6, 7]],
                    ins=[cc_in_v[:]],
                    outs=[cc_out_v[:]],
                )

            # ---- Q projection (collective mode: after V so collectives overlap) ----
            if collective:
                for qtr, tin in proj_quarters(qT, SQ):
                    for half in range(2):
                        cb = qtr * 2 + half
                        proj512(wq_sb, tin, half, QT[:, cb * 512:(cb + 1) * 512])

            # ---- masks ----
            mska_sb = constp.tile([128, 128], BF16)
            nc.sync.dma_start(mska_sb[:], mska[:])
            mskb_sb = constp.tile([128, 128], BF16)
            nc.sync.dma_start(mskb_sb[:], mskb[:])
            zbias = constp.tile([128, 1], F32)
            nc.vector.memset(zbias[:], 0.0)

            # ---- unpack gathered K/V ----
            if collective:
                for r in range(2):
                    nc.sync.dma_start(
                        KT[:, r * SK:(r + 1) * SK], cc_out_k[r]
                    )
                    nc.sync.dma_start(
                        VA[:, r * nloc_kt:(r + 1) * nloc_kt, 0:DV],
                        cc_out_v[r].rearrange("p (kt d) -> p kt d", d=DV),
                    )

            # ---- attention, per q-chunk of 512 ----
            for cc in range(NCH):
                npair = 4 * cc + 4
                PT = ptp.tile([128, MAXKT, 512], BF16, tag="pt")
                for a in range(npair):
                    j0 = max(0, a - 4 * cc)
                    n = (4 - j0) * 128
                    qoff = cc * 512 + j0 * 128
                    st = pstp.tile([128, 2, 512], F32, tag="stps")
                    for half in range(2):
                        kt = 2 * a + half
                        nc.tensor.matmul(
                            st[:, half, :n],
                            KT[:, kt * 128:(kt + 1) * 128],
                            QT[:, qoff:qoff + n],
                            start=True, stop=True,
                        )
                    nc.scalar.activation(
                        PT[:, 2 * a:2 * a + 2, j0 * 128:512],
                        st[:, :, :n],
                        Exp, bias=zbias[:],
                    )
                    for j in range(j0, 4):
                        i = 4 * cc + j
                        for half in range(2):
                            kt = 2 * a + half
                            msk = None
                            if kt == 2 * i:
                                msk = mska_sb
                            elif kt == 2 * i + 1:
                                msk = mskb_sb
                            if msk is not None:
                                sl = PT[:, kt, j * 128:(j + 1) * 128]
                                nc.vector.tensor_mul(sl, sl, msk[:])

                for j in range(4):
                    i = 4 * cc + j
                    nkt_i = 2 * i + 2
                    po = psop.tile([128, DV + 1], F32, tag="ops")
                    for kt in range(nkt_i):
                        nc.tensor.matmul(
                            po[:], PT[:, kt, j * 128:(j + 1) * 128], VA[:, kt, 0:DV + 1],
                            start=(kt == 0), stop=(kt == nkt_i - 1),
                        )
                    rec = outp.tile([128, 1], F32, tag="rec")
                    nc.vector.reciprocal(rec[:], po[:, DV:DV + 1])
                    ob = outp.tile([128, DV], F32, tag="ob")
                    nc.vector.tensor_scalar_mul(ob[:], po[:, 0:DV], rec[:])
                    nc.sync.dma_start(out[i * 128:(i + 1) * 128, :], ob[:])

    nc.compile()
    return nc


def _prep_inputs(q, k, v, W_Q, W_K, W_V, mode=None):
    mode = MODE if mode is None else mode
    collective = mode == "v2"
    q = np.asarray(q, dtype=np.float32)
    k = np.asarray(k, dtype=np.float32)
    v = np.asarray(v, dtype=np.float32)
    W_Q = np.asarray(W_Q, dtype=np.float32)
    W_K = np.asarray(W_K, dtype=np.float32)
    W_V = np.asarray(W_V, dtype=np.float32)

    scale = 1.0 / math.sqrt(DK)

    def wtile(w):  # [128, 1024] pre-tiled: row p, cols mc*128+d
        return np.ascontiguousarray(
            w.T.reshape(NMC, 128, w.shape[0]).transpose(1, 0, 2).reshape(128, NMC * w.shape[0])
        ).astype(BF16NP)

    wq_h = wtile(W_Q * scale)
    wk_h = wtile(W_K)
    wv_h = wtile(W_V)
    tri = np.triu(np.ones((128, 128), np.float32)).astype(BF16NP)
    ones = np.ones((128, 128), BF16NP)
    zeros = np.zeros((128, 128), BF16NP)

    in_maps = []
    for c in range(8):
        b, p = c // 2, c % 2
        idx = np.arange(NQT) * 2 + p
        qsel = q[b].reshape(MAXKT, 128, DM)[idx].reshape(SQ, DM)
        if collective:
            kslc = k[b, p * SK:(p + 1) * SK, :]
            vslc = v[b, p * SK:(p + 1) * SK, :]
        else:
            kslc = k[b]
            vslc = v[b]
        in_maps.append({
            "qT": np.ascontiguousarray(qsel.T).astype(BF16NP),
            "kT": np.ascontiguousarray(kslc.T).astype(BF16NP),
            "vT": np.ascontiguousarray(vslc.T).astype(BF16NP),
            "wq": wq_h, "wk": wk_h, "wv": wv_h,
            "mska": ones if p == 1 else tri,
            "mskb": tri if p == 1 else zeros,
            "eye": np.eye(128, dtype=np.float32).astype(BF16NP),
        })
    return in_maps


def kernel(q, k, v, W_Q, W_K, W_V):
    global LAST_RESULTS
    if MODE not in _NC_CACHE:
        _NC_CACHE[MODE] = build_nc(MODE)
    nc = _NC_CACHE[MODE]

    in_maps = _prep_inputs(q, k, v, W_Q, W_K, W_V)
    res = run_bass_kernel_spmd(nc, in_maps, core_ids=list(range(8)))
    LAST_RESULTS = res

    out = np.empty((B, S, DV), np.float32)
    for c in range(8):
        b, p = c // 2, c % 2
        oc = res.results[c]["out"]
        out[b].reshape(MAXKT, 128, DV)[np.arange(NQT) * 2 + p] = (
            oc.reshape(NQT, 128, DV)
        )
    return out

